# revision 12
# baseline (speedup 1.0000x reference)
"""DiT attention (B=2, S=2048, H=2048, 16 heads / 4 KV heads, RoPE) on 8 trn2
NeuronCores.

Sharding: core c -> batch b = c//4, head-group g = c%4 (q-heads 4g..4g+3 and
kv-head g).  Each core computes its heads' attention for its batch plus the
partial output projection over its 512 O-columns; the host sums the 4 partials
per batch and adds the output bias.

v4:
- phase-2 score/exp tiles are PAIRED into [128,1024] two-bank PSUM tiles so
  the ACT engine runs 8 exps per slot instead of 16 (per-instruction
  overhead was making ACT the pacer at 830ns/tile).
- softmax denominator: bf16 DVE pair-adds, then gpsimd partition_all_reduce
  (which also broadcasts), so normalization never touches the PE and is
  deferred into the next slot -- no head-of-line PE stalls.
- all weight / activation inputs are host-prepacked into the exact SBUF
  layout, so every load is a single wide-line contiguous DMA (the v2
  rearrange loads ran at ~30GB/s on 430B packets and starved the PE for
  40us at startup).
- x / Wq / Wk / Wv / exp-tiles / V are bfloat16 (matmul rate is identical
  to fp32r, but DMA bytes halve and the DVE denominator adds run in 2x
  mode); scores/rope/o-proj stay fp32(r).
- phase 1 uses 512-wide chunks so LDWEIGHTS hides under the moving stream;
  K is projected first per chunk.
- softmax denominator: exp tiles are summed over k-tiles on the DVE in two
  independent bf16 chains, then two 512-row PSUM-accumulated matmuls
  against the ones column do the partition reduction.
- phase-3 o-proj tiles interleave INSIDE the attention k-loop; their PSUM
  drains split between ACT and DVE (gpsimd cannot touch PSUM).
"""
import sys

if '/opt/trn_rl_repo' not in sys.path:
    sys.path.insert(0, '/opt/trn_rl_repo')

from contextlib import ExitStack

import numpy as np
import ml_dtypes

import concourse.bass as bass  # noqa: F401  (registers types)
import concourse.bass_isa as bass_isa
import concourse.tile as tile
import concourse.mybir as mybir
from concourse import bacc, bass_utils

B, S, H = 2, 2048, 2048
NH, NKV, HD = 16, 4, 128
P = 128
SCALING = HD ** -0.5
KO = H // P          # 16 contraction tiles for the projections
CH = 512             # sequence chunk (phase 1) and q/y chunk (phase 2/3)
NCH = S // CH        # 4
NQH = NH // NKV      # 4 q heads per core
KT = S // P          # 16 key tiles
F32 = mybir.dt.float32
F32R = mybir.dt.float32r
BF16 = mybir.dt.bfloat16
AF = mybir.ActivationFunctionType
BF16NP = ml_dtypes.bfloat16

_NC_CACHE = []


def _build_nc():
    nc = bacc.Bacc("TRN2", target_bir_lowering=False, debug=False,
                   enable_asserts=True, num_devices=8)
    # all inputs are host-prepacked to SBUF layout (partition dim first)
    xt = nc.dram_tensor("xt", [P, NCH, KO, CH], BF16,
                        kind="ExternalInput").ap()
    wq = nc.dram_tensor("wq", [P, KO, NQH * HD], BF16,
                        kind="ExternalInput").ap()
    wk = nc.dram_tensor("wk", [P, KO, HD], BF16, kind="ExternalInput").ap()
    wv = nc.dram_tensor("wv", [P, KO, HD], BF16, kind="ExternalInput").ap()
    wo = nc.dram_tensor("wo", [P, NQH, H], F32R, kind="ExternalInput").ap()
    cosT = nc.dram_tensor("cosT", [HD, S], F32, kind="ExternalInput").ap()
    sinT = nc.dram_tensor("sinT", [HD, S], F32, kind="ExternalInput").ap()
    bqT = nc.dram_tensor("bqT", [HD, NQH], F32, kind="ExternalInput").ap()
    bkT = nc.dram_tensor("bkT", [HD, 1], F32, kind="ExternalInput").ap()
    bvT = nc.dram_tensor("bvT", [HD, 1], F32, kind="ExternalInput").ap()
    # [:, 0:128] identity (PE transpose), [:, 128] all-ones (denominator)
    ones = nc.dram_tensor("ones", [P, P + 1], BF16, kind="ExternalInput").ap()
    y = nc.dram_tensor("y", [S, H], F32, kind="ExternalOutput").ap()

    with tile.TileContext(nc) as tc, ExitStack() as ctx:
        const = ctx.enter_context(tc.tile_pool(name="const", bufs=1))
        bq_sb = const.tile([HD, NQH], F32)
        bk_sb = const.tile([HD, 1], F32)
        bv_sb = const.tile([HD, 1], F32)
        on_sb = const.tile([P, P + 1], BF16)
        nc.sync.dma_start(bq_sb[:], bqT)
        nc.sync.dma_start(bk_sb[:], bkT)
        nc.sync.dma_start(bv_sb[:], bvT)
        nc.sync.dma_start(on_sb[:], ones)

        res = ctx.enter_context(tc.tile_pool(name="res", bufs=1))
        qrop = res.tile([HD, NQH, S], F32R)   # Q^T roped; reused as O^T later
        krop = res.tile([HD, S], F32R)        # K^T roped
        v_sb = res.tile([P, KT, HD], BF16)    # V natural, k-tiled

        # wo lives beside the phase-1 pools (bf16 front end frees the room)
        wop = ctx.enter_context(tc.tile_pool(name="wop", bufs=1))
        wo_sb = wop.tile([P, NQH, H], F32R)

        p1_stack = ExitStack()
        wp = p1_stack.enter_context(tc.tile_pool(name="wp", bufs=1))
        wq_sb = wp.tile([P, KO, NQH * HD], BF16)
        wk_sb = wp.tile([P, KO, HD], BF16)
        wv_sb = wp.tile([P, KO, HD], BF16)
        csp = p1_stack.enter_context(tc.tile_pool(name="csp", bufs=1))
        cos_sb = csp.tile([HD, S], F32)
        sin_sb = csp.tile([HD, S], F32)
        xtp = p1_stack.enter_context(tc.tile_pool(name="xtp", bufs=3))
        p1s = p1_stack.enter_context(tc.tile_pool(name="p1s", bufs=3))
        p1ps = p1_stack.enter_context(
            tc.tile_pool(name="p1ps", bufs=2, space="PSUM"))
        tpp = p1_stack.enter_context(
            tc.tile_pool(name="tpp", bufs=2, space="PSUM"))

        # weight DMAs in first-use order; all contiguous wide-line loads.
        nc.sync.dma_start(wk_sb[:], wk)
        nc.sync.dma_start(wq_sb[:], wq)
        nc.sync.dma_start(wv_sb[:], wv)
        for c in range(NCH):
            scol = slice(c * CH, (c + 1) * CH)
            nc.scalar.dma_start(cos_sb[:, scol], cosT[:, scol])
            nc.scalar.dma_start(sin_sb[:, scol], sinT[:, scol])
        # wo last: it is not needed until ~100us in; keep startup bandwidth
        # for xt / weights / tables
        nc.scalar.dma_start(wo_sb[:], wo)

        # ---------------- phase 1: projections + RoPE ----------------
        # per chunk: K first (so krop completes earliest), then Q0..Q3, V.
        for c in range(NCH):
            scol = slice(c * CH, (c + 1) * CH)
            xt_c = xtp.tile([P, KO, CH], BF16, tag="xt")
            # paced ko groups so the first matmul can start early
            ko0 = 0
            for gsz in (1, 1, 2, 4, 8):
                ksl = slice(ko0, ko0 + gsz)
                nc.gpsimd.dma_start(xt_c[:, ksl, :], xt[:, c, ksl, :])
                ko0 += gsz
            # order: K, Q0..3, V
            for which in range(NQH + 2):
                ps = p1ps.tile([P, CH], F32, tag="p")
                for ko in range(KO):
                    if which == 0:
                        lhsT = wk_sb[:, ko, :]
                    elif which <= NQH:
                        h = which - 1
                        lhsT = wq_sb[:, ko, h * HD:(h + 1) * HD]
                    else:
                        lhsT = wv_sb[:, ko, :]
                    nc.tensor.matmul(ps[:], lhsT, xt_c[:, ko, :],
                                     start=(ko == 0), stop=(ko == KO - 1))
                if which <= NQH:
                    bias = bk_sb[:, 0:1] if which == 0 \
                        else bq_sb[:, which - 1:which]
                    qf = p1s.tile([HD, CH], F32, tag="qf")
                    nc.scalar.activation(qf[:], ps[:], AF.Identity,
                                         bias=bias, scale=1.0)
                    qs = p1s.tile([HD, CH], F32, tag="qs")
                    nc.sync.dma_start(qs[0:64, :], qf[64:128, :])
                    nc.sync.dma_start(qs[64:128, :], qf[0:64, :])
                    t2 = p1s.tile([HD, CH], F32, tag="t2")
                    nc.vector.tensor_mul(t2[:], qs[:], sin_sb[:, scol])
                    nc.vector.tensor_mul(qf[:], qf[:], cos_sb[:, scol])
                    dst = krop[:, scol] if which == 0 \
                        else qrop[:, which - 1, scol]
                    nc.vector.tensor_add(dst, qf[:], t2[:])
                else:
                    # V^T -> (bias-add) -> PE-transpose to V natural (bf16)
                    vt = p1s.tile([HD, CH], BF16, tag="vt")
                    nc.scalar.activation(vt[:], ps[:], AF.Identity,
                                         bias=bv_sb[:, 0:1], scale=1.0)
                    for st in range(CH // P):
                        kt = c * (CH // P) + st
                        tps = tpp.tile([P, P], BF16, tag="tp")
                        nc.tensor.transpose(
                            tps[:], vt[:, st * P:(st + 1) * P],
                            on_sb[:, 0:P])
                        nc.vector.tensor_copy(v_sb[:, kt, :], tps[:])

        # release phase-1 pools; later pools reuse their space
        p1_stack.close()

        # ---------------- phase 2 + 3 interleaved ----------------
        NP2 = KT // 2          # 8 score/exp pairs per slot
        p2s = ctx.enter_context(tc.tile_pool(name="p2s", bufs=3))
        dap = ctx.enter_context(tc.tile_pool(name="dap", bufs=2))
        p2sm = ctx.enter_context(tc.tile_pool(name="p2sm", bufs=2))
        stp = ctx.enter_context(tc.tile_pool(name="stp", bufs=2, space="PSUM"))
        opp = ctx.enter_context(tc.tile_pool(name="opp", bufs=2, space="PSUM"))
        p3s = ctx.enter_context(tc.tile_pool(name="p3s", bufs=3))
        ypp = ctx.enter_context(tc.tile_pool(name="yp", bufs=2, space="PSUM"))

        p3q = []  # pending (qt, ycn) output-projection tiles
        p3n = [0]  # emitted-tile counter (alternates the PSUM-drain engine)

        def emit_p3_tile(qt, ycn):
            ysl = slice(ycn * CH, (ycn + 1) * CH)
            y_ps = ypp.tile([P, CH], F32, tag="y")
            for hh in range(NQH):
                nc.tensor.matmul(y_ps[:],
                                 qrop[:, hh, qt * P:(qt + 1) * P],
                                 wo_sb[:, hh, ysl],
                                 start=(hh == 0), stop=(hh == NQH - 1))
            y_sb = p3s.tile([P, CH], F32, tag="ysb")
            # gpsimd can't read PSUM; split drains between ACT and DVE
            if p3n[0] % 2 == 0:
                nc.scalar.copy(y_sb[:], y_ps[:])
            else:
                nc.vector.tensor_copy(y_sb[:], y_ps[:])
            p3n[0] += 1
            nc.sync.dma_start(y[qt * P:(qt + 1) * P, ysl], y_sb[:])

        def emit_normalize(prev):
            # Entirely PE-free: gpsimd all-reduce (includes the broadcast),
            # then 3 DVE ops.  o-mul lands a few us into the next slot.
            da, o_ps, hp, qslp = prev
            dred = p2sm.tile([P, 2 * CH], F32, tag="dred")
            nc.gpsimd.partition_all_reduce(dred[:], da[:], P,
                                           bass_isa.ReduceOp.add)
            dsum = p2sm.tile([P, CH], F32, tag="dsum")
            nc.vector.tensor_add(dsum[:], dred[:, 0:CH], dred[:, CH:2 * CH])
            rec = p2sm.tile([P, CH], F32, tag="rec")
            nc.vector.reciprocal_approx_fast(rec[:], dsum[:])
            # normalized O^T overwrites the spent Q^T slice
            nc.vector.tensor_mul(qrop[:, hp, qslp], o_ps[:], rec[:])

        def score_pair(pr, h, qsl):
            st2 = stp.tile([P, 2 * CH], F32, tag="st", name="st")
            for j in range(2):
                kt = 2 * pr + j
                nc.tensor.matmul(st2[:, j * CH:(j + 1) * CH],
                                 krop[:, kt * P:(kt + 1) * P],
                                 qrop[:, h, qsl], start=True, stop=True)
            return st2

        prev = None
        for qc in range(NCH):
            qsl = slice(qc * CH, (qc + 1) * CH)
            for h in range(NQH):
                o_ps = opp.tile([HD, CH], F32, tag="o")
                da = dap.tile([P, 2 * CH], BF16, tag="da")
                st2 = score_pair(0, h, qsl)
                pt_prev = None
                for pr in range(NP2):
                    st_next = score_pair(pr + 1, h, qsl) if pr + 1 < NP2 \
                        else None
                    pt2 = p2s.tile([P, 2 * CH], BF16, tag="pt")
                    nc.scalar.activation(pt2[:], st2[:], AF.Exp,
                                         scale=SCALING)
                    st2 = st_next
                    for j in range(2):
                        kt = 2 * pr + j
                        nc.tensor.matmul(o_ps[:], v_sb[:, kt, :],
                                         pt2[:, j * CH:(j + 1) * CH],
                                         start=(kt == 0), stop=(kt == KT - 1))
                    if pr == 1:
                        nc.vector.tensor_add(da[:], pt_prev[:], pt2[:])
                        # deferred normalize of the previous head runs on
                        # gpsimd/DVE underneath this slot's PE work
                        if prev is not None:
                            emit_normalize(prev)
                            prev = None
                    elif pr > 1:
                        nc.vector.tensor_add(da[:], da[:], pt2[:])
                    pt_prev = pt2
                    # interleave one o-proj tile per mid-slot pair
                    if pr in (3, 4, 5, 6) and p3q:
                        emit_p3_tile(*p3q.pop(0))
                prev = (da, o_ps, h, qsl)
            p3q.extend([(qc * (CH // P) + i, ycn)
                        for i in range(CH // P) for ycn in range(NCH)])

        # final head's normalize, then drain remaining o-proj tiles
        emit_normalize(prev)
        for qt, ycn in p3q:
            emit_p3_tile(qt, ycn)

    nc.compile()
    return nc


def _get_nc():
    if not _NC_CACHE:
        _NC_CACHE.append(_build_nc())
    return _NC_CACHE[0]


def kernel(**inputs) -> np.ndarray:
    hs = np.asarray(inputs["hidden_states"], np.float32)
    cos = np.asarray(inputs["cos"], np.float32)
    sin = np.asarray(inputs["sin"], np.float32)
    Wq = np.asarray(inputs["Wq"], np.float32)
    bq = np.asarray(inputs["bq"], np.float32)
    Wk = np.asarray(inputs["Wk"], np.float32)
    bk = np.asarray(inputs["bk"], np.float32)
    Wv = np.asarray(inputs["Wv"], np.float32)
    bv = np.asarray(inputs["bv"], np.float32)
    Wo = np.asarray(inputs["Wo"], np.float32)
    bo = np.asarray(inputs["bo"], np.float32)

    nc = _get_nc()

    # host-side packing into SBUF layouts (wide-line contiguous DMAs)
    def pack_x(x):      # [S, H] -> [P, NCH, KO, CH] bf16
        return np.ascontiguousarray(
            x.reshape(NCH, CH, KO, P).transpose(3, 0, 2, 1).astype(BF16NP))

    def pack_w(w, m):   # [H, m] -> [P, KO, m] bf16
        return np.ascontiguousarray(
            w.reshape(KO, P, m).transpose(1, 0, 2).astype(BF16NP))

    def pack_wo(w):     # [NQH*HD, H] -> [P, NQH, H] f32
        return np.ascontiguousarray(w.reshape(NQH, P, H).transpose(1, 0, 2))

    XT = [pack_x(hs[b]) for b in range(B)]
    cosT = [np.ascontiguousarray(cos[b].T) for b in range(B)]
    sinTs = []
    for b in range(B):
        st = np.ascontiguousarray(sin[b].T)
        st[0:64] = -st[0:64]          # fold rotate_half sign into the table
        sinTs.append(st)
    ones = np.zeros((P, P + 1), np.float32)
    ones[:, 0:P] = np.eye(P, dtype=np.float32)
    ones[:, P] = 1.0
    ones = ones.astype(BF16NP)

    in_maps = []
    for c in range(8):
        b, g = c // 4, c % 4
        in_maps.append({
            "xt": XT[b],
            "wq": pack_w(Wq[:, g * NQH * HD:(g + 1) * NQH * HD], NQH * HD),
            "wk": pack_w(Wk[:, g * HD:(g + 1) * HD], HD),
            "wv": pack_w(Wv[:, g * HD:(g + 1) * HD], HD),
            "wo": pack_wo(Wo[g * NQH * HD:(g + 1) * NQH * HD, :]),
            "cosT": cosT[b],
            "sinT": sinTs[b],
            "bqT": np.ascontiguousarray(
                bq[g * NQH * HD:(g + 1) * NQH * HD].reshape(NQH, HD).T),
            "bkT": np.ascontiguousarray(
                bk[g * HD:(g + 1) * HD].reshape(1, HD).T),
            "bvT": np.ascontiguousarray(
                bv[g * HD:(g + 1) * HD].reshape(1, HD).T),
            "ones": ones,
        })

    res = bass_utils.run_bass_kernel_spmd(nc, in_maps, core_ids=list(range(8)))

    out = np.empty((B, S, H), np.float32)
    for b in range(B):
        acc = res.results[4 * b]["y"].copy()
        for g in range(1, 4):
            acc += res.results[4 * b + g]["y"]
        out[b] = acc + bo[None, :]
    return out


# revision 13
# speedup vs baseline: 1.3139x; 1.3139x over previous
"""DiT attention (B=2, S=2048, H=2048, 16 heads / 4 KV heads, RoPE) on 8 trn2
NeuronCores.

Sharding: core c -> batch b = c//4, head-group g = c%4 (q-heads 4g..4g+3 and
kv-head g).  Each core computes its heads' attention for its batch plus the
partial output projection over its 512 O-columns; the host sums the 4 partials
per batch and adds the output bias.

v4:
- phase-2 score/exp tiles are PAIRED into [128,1024] two-bank PSUM tiles so
  the ACT engine runs 8 exps per slot instead of 16 (per-instruction
  overhead was making ACT the pacer at 830ns/tile).
- softmax denominator: bf16 DVE pair-adds, then gpsimd partition_all_reduce
  (which also broadcasts), so normalization never touches the PE and is
  deferred into the next slot -- no head-of-line PE stalls.
- all weight / activation inputs are host-prepacked into the exact SBUF
  layout, so every load is a single wide-line contiguous DMA (the v2
  rearrange loads ran at ~30GB/s on 430B packets and starved the PE for
  40us at startup).
- x / Wq / Wk / Wv / exp-tiles / V are bfloat16 (matmul rate is identical
  to fp32r, but DMA bytes halve and the DVE denominator adds run in 2x
  mode); scores/rope/o-proj stay fp32(r).
- phase 1 uses 512-wide chunks so LDWEIGHTS hides under the moving stream;
  K is projected first per chunk.
- softmax denominator: exp tiles are summed over k-tiles on the DVE in two
  independent bf16 chains, then two 512-row PSUM-accumulated matmuls
  against the ones column do the partition reduction.
- phase-3 o-proj tiles interleave INSIDE the attention k-loop; their PSUM
  drains split between ACT and DVE (gpsimd cannot touch PSUM).
"""
import sys

if '/opt/trn_rl_repo' not in sys.path:
    sys.path.insert(0, '/opt/trn_rl_repo')

from contextlib import ExitStack

import numpy as np
import ml_dtypes

import concourse.bass as bass  # noqa: F401  (registers types)
import concourse.bass_isa as bass_isa
import concourse.tile as tile
import concourse.mybir as mybir
from concourse import bacc, bass_utils

B, S, H = 2, 2048, 2048
NH, NKV, HD = 16, 4, 128
P = 128
SCALING = HD ** -0.5
KO = H // P          # 16 contraction tiles for the projections
CH = 512             # sequence chunk (phase 1) and q/y chunk (phase 2/3)
NCH = S // CH        # 4
NQH = NH // NKV      # 4 q heads per core
KT = S // P          # 16 key tiles
F32 = mybir.dt.float32
F32R = mybir.dt.float32r
BF16 = mybir.dt.bfloat16
AF = mybir.ActivationFunctionType
BF16NP = ml_dtypes.bfloat16

_NC_CACHE = []


def _build_nc():
    nc = bacc.Bacc("TRN2", target_bir_lowering=False, debug=False,
                   enable_asserts=True, num_devices=8)
    # all inputs are host-prepacked to SBUF layout (partition dim first)
    xt = nc.dram_tensor("xt", [P, NCH, KO, CH], BF16,
                        kind="ExternalInput").ap()
    wq = nc.dram_tensor("wq", [P, KO, NQH * HD], BF16,
                        kind="ExternalInput").ap()
    wk = nc.dram_tensor("wk", [P, KO, HD], BF16, kind="ExternalInput").ap()
    wv = nc.dram_tensor("wv", [P, KO, HD], BF16, kind="ExternalInput").ap()
    wo = nc.dram_tensor("wo", [P, NQH, H], F32R, kind="ExternalInput").ap()
    cosT = nc.dram_tensor("cosT", [HD, S], F32, kind="ExternalInput").ap()
    sinT = nc.dram_tensor("sinT", [HD, S], F32, kind="ExternalInput").ap()
    bqT = nc.dram_tensor("bqT", [HD, NQH], F32, kind="ExternalInput").ap()
    bkT = nc.dram_tensor("bkT", [HD, 1], F32, kind="ExternalInput").ap()
    bvT = nc.dram_tensor("bvT", [HD, 1], F32, kind="ExternalInput").ap()
    # [:, 0:128] identity (PE transpose), [:, 128] all-ones (denominator)
    ones = nc.dram_tensor("ones", [P, P + 1], BF16, kind="ExternalInput").ap()
    y = nc.dram_tensor("y", [S, H], F32, kind="ExternalOutput").ap()

    with tile.TileContext(nc) as tc, ExitStack() as ctx:
        const = ctx.enter_context(tc.tile_pool(name="const", bufs=1))
        bq_sb = const.tile([HD, NQH], F32)
        bk_sb = const.tile([HD, 1], F32)
        bv_sb = const.tile([HD, 1], F32)
        on_sb = const.tile([P, P + 1], BF16)
        nc.sync.dma_start(bq_sb[:], bqT)
        nc.sync.dma_start(bk_sb[:], bkT)
        nc.sync.dma_start(bv_sb[:], bvT)
        nc.sync.dma_start(on_sb[:], ones)

        res = ctx.enter_context(tc.tile_pool(name="res", bufs=1))
        qrop = res.tile([HD, NQH, S], F32R)   # Q^T roped; reused as O^T later
        krop = res.tile([HD, S], F32R)        # K^T roped
        v_sb = res.tile([P, KT, HD], BF16)    # V natural, k-tiled

        # wo lives beside the phase-1 pools (bf16 front end frees the room)
        wop = ctx.enter_context(tc.tile_pool(name="wop", bufs=1))
        wo_sb = wop.tile([P, NQH, H], F32R)

        p1_stack = ExitStack()
        wp = p1_stack.enter_context(tc.tile_pool(name="wp", bufs=1))
        wq_sb = wp.tile([P, KO, NQH * HD], BF16)
        wk_sb = wp.tile([P, KO, HD], BF16)
        wv_sb = wp.tile([P, KO, HD], BF16)
        csp = p1_stack.enter_context(tc.tile_pool(name="csp", bufs=1))
        cos_sb = csp.tile([HD, S], F32)
        sin_sb = csp.tile([HD, S], F32)
        xtp = p1_stack.enter_context(tc.tile_pool(name="xtp", bufs=3))
        p1s = p1_stack.enter_context(tc.tile_pool(name="p1s", bufs=3))
        p1ps = p1_stack.enter_context(
            tc.tile_pool(name="p1ps", bufs=2, space="PSUM"))
        tpp = p1_stack.enter_context(
            tc.tile_pool(name="tpp", bufs=2, space="PSUM"))

        # weight DMAs in first-use order; all contiguous wide-line loads.
        nc.sync.dma_start(wk_sb[:], wk)
        nc.sync.dma_start(wq_sb[:], wq)
        nc.sync.dma_start(wv_sb[:], wv)
        for c in range(NCH):
            scol = slice(c * CH, (c + 1) * CH)
            nc.scalar.dma_start(cos_sb[:, scol], cosT[:, scol])
            nc.scalar.dma_start(sin_sb[:, scol], sinT[:, scol])
        # wo last: it is not needed until ~100us in; keep startup bandwidth
        # for xt / weights / tables
        nc.scalar.dma_start(wo_sb[:], wo)

        # ---------------- phase 1: projections + RoPE ----------------
        # per chunk: K first (so krop completes earliest), then Q0..Q3, V.
        for c in range(NCH):
            scol = slice(c * CH, (c + 1) * CH)
            xt_c = xtp.tile([P, KO, CH], BF16, tag="xt")
            # paced ko groups so the first matmul can start early
            ko0 = 0
            for gsz in (1, 1, 2, 4, 8):
                ksl = slice(ko0, ko0 + gsz)
                nc.gpsimd.dma_start(xt_c[:, ksl, :], xt[:, c, ksl, :])
                ko0 += gsz
            # order: K, Q0..3, V
            for which in range(NQH + 2):
                ps = p1ps.tile([P, CH], F32, tag="p")
                for ko in range(KO):
                    if which == 0:
                        lhsT = wk_sb[:, ko, :]
                    elif which <= NQH:
                        h = which - 1
                        lhsT = wq_sb[:, ko, h * HD:(h + 1) * HD]
                    else:
                        lhsT = wv_sb[:, ko, :]
                    nc.tensor.matmul(ps[:], lhsT, xt_c[:, ko, :],
                                     start=(ko == 0), stop=(ko == KO - 1))
                if which <= NQH:
                    bias = bk_sb[:, 0:1] if which == 0 \
                        else bq_sb[:, which - 1:which]
                    qf = p1s.tile([HD, CH], F32, tag="qf")
                    nc.scalar.activation(qf[:], ps[:], AF.Identity,
                                         bias=bias, scale=1.0)
                    qs = p1s.tile([HD, CH], F32, tag="qs")
                    nc.sync.dma_start(qs[0:64, :], qf[64:128, :])
                    nc.sync.dma_start(qs[64:128, :], qf[0:64, :])
                    t2 = p1s.tile([HD, CH], F32, tag="t2")
                    nc.vector.tensor_mul(t2[:], qs[:], sin_sb[:, scol])
                    nc.vector.tensor_mul(qf[:], qf[:], cos_sb[:, scol])
                    dst = krop[:, scol] if which == 0 \
                        else qrop[:, which - 1, scol]
                    nc.vector.tensor_add(dst, qf[:], t2[:])
                else:
                    # V^T -> (bias-add) -> PE-transpose to V natural (bf16)
                    vt = p1s.tile([HD, CH], BF16, tag="vt")
                    nc.scalar.activation(vt[:], ps[:], AF.Identity,
                                         bias=bv_sb[:, 0:1], scale=1.0)
                    for st in range(CH // P):
                        kt = c * (CH // P) + st
                        tps = tpp.tile([P, P], BF16, tag="tp")
                        nc.tensor.transpose(
                            tps[:], vt[:, st * P:(st + 1) * P],
                            on_sb[:, 0:P])
                        nc.vector.tensor_copy(v_sb[:, kt, :], tps[:])

        # release phase-1 pools; later pools reuse their space
        p1_stack.close()

        # ---------------- phase 2 + 3 interleaved ----------------
        NP2 = KT // 2          # 8 score/exp pairs per slot
        p2s = ctx.enter_context(tc.tile_pool(name="p2s", bufs=3))
        dap = ctx.enter_context(tc.tile_pool(name="dap", bufs=2))
        p2sm = ctx.enter_context(tc.tile_pool(name="p2sm", bufs=2))
        stp = ctx.enter_context(tc.tile_pool(name="stp", bufs=2, space="PSUM"))
        opp = ctx.enter_context(tc.tile_pool(name="opp", bufs=2, space="PSUM"))
        p3s = ctx.enter_context(tc.tile_pool(name="p3s", bufs=3))
        ypp = ctx.enter_context(tc.tile_pool(name="yp", bufs=2, space="PSUM"))

        p3q = []  # pending (qt, ycn) output-projection tiles
        p3n = [0]  # emitted-tile counter (alternates the PSUM-drain engine)

        def emit_p3_tile(qt, ycn):
            ysl = slice(ycn * CH, (ycn + 1) * CH)
            y_ps = ypp.tile([P, CH], F32, tag="y")
            for hh in range(NQH):
                nc.tensor.matmul(y_ps[:],
                                 qrop[:, hh, qt * P:(qt + 1) * P],
                                 wo_sb[:, hh, ysl],
                                 start=(hh == 0), stop=(hh == NQH - 1))
            y_sb = p3s.tile([P, CH], F32, tag="ysb")
            # gpsimd can't read PSUM; split drains between ACT and DVE
            if p3n[0] % 2 == 0:
                nc.scalar.copy(y_sb[:], y_ps[:])
            else:
                nc.vector.tensor_copy(y_sb[:], y_ps[:])
            p3n[0] += 1
            nc.sync.dma_start(y[qt * P:(qt + 1) * P, ysl], y_sb[:])

        def emit_normalize(prev):
            # Two 512-row PSUM-accumulated matmuls against the ones column
            # reduce the two k-parity chains across partitions.  The [1,CH]
            # output borrows a ypp-pool tile (PSUM budget stays at 8 banks).
            # Deferred into the next slot, so nothing here stalls the PE.
            da, o_ps, hp, qslp = prev
            s_t = ypp.tile([P, CH], F32, tag="y", name="s_t")
            nc.tensor.matmul(s_t[0:1, :], on_sb[:, P:P + 1], da[:, 0:CH],
                             start=True, stop=False)
            nc.tensor.matmul(s_t[0:1, :], on_sb[:, P:P + 1], da[:, CH:2 * CH],
                             start=False, stop=True)
            rec = p2sm.tile([1, CH], F32, tag="rec")
            nc.vector.reciprocal_approx_fast(rec[:], s_t[0:1, :])
            rb = p2sm.tile([P, CH], F32, tag="rb")
            nc.gpsimd.partition_broadcast(rb[:], rec[:])
            # normalized O^T overwrites the spent Q^T slice
            nc.vector.tensor_mul(qrop[:, hp, qslp], o_ps[:], rb[:])

        def score_pair(pr, h, qsl):
            st2 = stp.tile([P, 2 * CH], F32, tag="st", name="st")
            for j in range(2):
                kt = 2 * pr + j
                nc.tensor.matmul(st2[:, j * CH:(j + 1) * CH],
                                 krop[:, kt * P:(kt + 1) * P],
                                 qrop[:, h, qsl], start=True, stop=True)
            return st2

        prev = None
        for qc in range(NCH):
            qsl = slice(qc * CH, (qc + 1) * CH)
            for h in range(NQH):
                o_ps = opp.tile([HD, CH], F32, tag="o")
                da = dap.tile([P, 2 * CH], BF16, tag="da")
                st2 = score_pair(0, h, qsl)
                pt_prev = None
                for pr in range(NP2):
                    st_next = score_pair(pr + 1, h, qsl) if pr + 1 < NP2 \
                        else None
                    pt2 = p2s.tile([P, 2 * CH], BF16, tag="pt")
                    nc.scalar.activation(pt2[:], st2[:], AF.Exp,
                                         scale=SCALING)
                    st2 = st_next
                    for j in range(2):
                        kt = 2 * pr + j
                        nc.tensor.matmul(o_ps[:], v_sb[:, kt, :],
                                         pt2[:, j * CH:(j + 1) * CH],
                                         start=(kt == 0), stop=(kt == KT - 1))
                    if pr == 1:
                        nc.vector.tensor_add(da[:], pt_prev[:], pt2[:])
                        # deferred normalize of the previous head runs on
                        # gpsimd/DVE underneath this slot's PE work
                        if prev is not None:
                            emit_normalize(prev)
                            prev = None
                    elif pr > 1:
                        nc.vector.tensor_add(da[:], da[:], pt2[:])
                    pt_prev = pt2
                    # interleave one o-proj tile per mid-slot pair
                    if pr in (3, 4, 5, 6) and p3q:
                        emit_p3_tile(*p3q.pop(0))
                prev = (da, o_ps, h, qsl)
            p3q.extend([(qc * (CH // P) + i, ycn)
                        for i in range(CH // P) for ycn in range(NCH)])

        # final head's normalize, then drain remaining o-proj tiles
        emit_normalize(prev)
        for qt, ycn in p3q:
            emit_p3_tile(qt, ycn)

    nc.compile()
    return nc


def _get_nc():
    if not _NC_CACHE:
        _NC_CACHE.append(_build_nc())
    return _NC_CACHE[0]


def kernel(**inputs) -> np.ndarray:
    hs = np.asarray(inputs["hidden_states"], np.float32)
    cos = np.asarray(inputs["cos"], np.float32)
    sin = np.asarray(inputs["sin"], np.float32)
    Wq = np.asarray(inputs["Wq"], np.float32)
    bq = np.asarray(inputs["bq"], np.float32)
    Wk = np.asarray(inputs["Wk"], np.float32)
    bk = np.asarray(inputs["bk"], np.float32)
    Wv = np.asarray(inputs["Wv"], np.float32)
    bv = np.asarray(inputs["bv"], np.float32)
    Wo = np.asarray(inputs["Wo"], np.float32)
    bo = np.asarray(inputs["bo"], np.float32)

    nc = _get_nc()

    # host-side packing into SBUF layouts (wide-line contiguous DMAs)
    def pack_x(x):      # [S, H] -> [P, NCH, KO, CH] bf16
        return np.ascontiguousarray(
            x.reshape(NCH, CH, KO, P).transpose(3, 0, 2, 1).astype(BF16NP))

    def pack_w(w, m):   # [H, m] -> [P, KO, m] bf16
        return np.ascontiguousarray(
            w.reshape(KO, P, m).transpose(1, 0, 2).astype(BF16NP))

    def pack_wo(w):     # [NQH*HD, H] -> [P, NQH, H] f32
        return np.ascontiguousarray(w.reshape(NQH, P, H).transpose(1, 0, 2))

    XT = [pack_x(hs[b]) for b in range(B)]
    cosT = [np.ascontiguousarray(cos[b].T) for b in range(B)]
    sinTs = []
    for b in range(B):
        st = np.ascontiguousarray(sin[b].T)
        st[0:64] = -st[0:64]          # fold rotate_half sign into the table
        sinTs.append(st)
    ones = np.zeros((P, P + 1), np.float32)
    ones[:, 0:P] = np.eye(P, dtype=np.float32)
    ones[:, P] = 1.0
    ones = ones.astype(BF16NP)

    in_maps = []
    for c in range(8):
        b, g = c // 4, c % 4
        in_maps.append({
            "xt": XT[b],
            "wq": pack_w(Wq[:, g * NQH * HD:(g + 1) * NQH * HD], NQH * HD),
            "wk": pack_w(Wk[:, g * HD:(g + 1) * HD], HD),
            "wv": pack_w(Wv[:, g * HD:(g + 1) * HD], HD),
            "wo": pack_wo(Wo[g * NQH * HD:(g + 1) * NQH * HD, :]),
            "cosT": cosT[b],
            "sinT": sinTs[b],
            "bqT": np.ascontiguousarray(
                bq[g * NQH * HD:(g + 1) * NQH * HD].reshape(NQH, HD).T),
            "bkT": np.ascontiguousarray(
                bk[g * HD:(g + 1) * HD].reshape(1, HD).T),
            "bvT": np.ascontiguousarray(
                bv[g * HD:(g + 1) * HD].reshape(1, HD).T),
            "ones": ones,
        })

    res = bass_utils.run_bass_kernel_spmd(nc, in_maps, core_ids=list(range(8)))

    out = np.empty((B, S, H), np.float32)
    for b in range(B):
        acc = res.results[4 * b]["y"].copy()
        for g in range(1, 4):
            acc += res.results[4 * b + g]["y"]
        out[b] = acc + bo[None, :]
    return out
